# revision 64
# baseline (speedup 1.0000x reference)
"""Multi-head attention (b=2, n=2048, d_model=1024, H=16, d_k=d_v=64) on 8
Trainium2 NeuronCores.

Sharding: 8 cores = 2 (batch) x 4 (head groups of 4 heads).  Each core
computes, for its batch ib and head group g (heads as 2 pairs x 2 hp):

    qT/kT projections   qT = Wq_g @ x^T            [256, 2048]
    v projection        V  = x @ Wv_g^T            [2048, 256]
    per head: S^T = K_h Q_h^T (kpos on partitions), A^T = exp(S^T/8),
              O^T|Z = [V_h|1]^T A^T  (PSUM row 64 gives softmax denom Z)
    normalize O^T by 1/Z (Z broadcast across partitions via a tiny
    ones-outer-product matmul on the PE -- no DRAM roundtrip),
    out-projection Y^T = Wo_g @ O_cat^T            [1024, 2048]

Host sums the 4 per-group partial Y^T per batch and adds bo + Wo@bv
(the V bias commutes through normalized attention; the K bias is a
per-query constant shift of the logits, to which softmax is exactly
invariant, so both are folded out of the device program).

All matmul operands are bf16 (PSUM accumulation stays fp32): this halves
input DMA, enables Fast Weight Load on the PE (2x faster LDWEIGHTS), and
doubles DVE throughput where it applies.  exp is evaluated on fp32 PSUM
logits.  Softmax skips max-subtraction: scores*scale are ~N(0,1).

The PE instruction stream is software-pipelined: within the attention
inner loop the AV matmuls lag the S matmuls by one k-tile so the ACT
(exp) stream is hidden, and projection / out-projection matmuls are
sprinkled between attention tiles as filler so the PE never idles long
enough for the HAM clock gate to re-throttle.
"""

import numpy as np
from contextlib import ExitStack

import ml_dtypes

import concourse.bass as bass
import concourse.mybir as mybir
import concourse.tile as tile
from concourse import bacc
from concourse.bass_utils import run_bass_kernel_spmd

F32 = mybir.dt.float32
BF16 = mybir.dt.bfloat16
NP_BF16 = ml_dtypes.bfloat16
EXP = mybir.ActivationFunctionType.Exp
ADD = mybir.AluOpType.add
MULT = mybir.AluOpType.mult

D_MODEL = 1024
H = 16
DK = 64
B = 2
N = 2048           # nq = nk
G = 4              # head groups (cores per batch)
HG = H // G        # heads per group = 4
DG = HG * DK       # 256 group dims
KT = 8             # D_MODEL / 128 contraction tiles
NKT = N // 128     # 16 k-tiles in attention
QB = 512           # attention q-block
NQB = N // QB      # 4 q-blocks
P = 128

_PROGRAM = None
DEBUG_DUMP = False


def _build_program():
    nc = bacc.Bacc("TRN2", target_bir_lowering=False, debug=False, num_devices=8)

    # pre-tiled on host (bf16): per-partition lines are contiguous reads
    xqT = nc.dram_tensor("xqT", [NQB, P, KT, QB], BF16, kind="ExternalInput").ap()
    xkT = nc.dram_tensor("xkT", [NQB, P, KT, QB], BF16, kind="ExternalInput").ap()
    xvT = nc.dram_tensor("xvT", [NKT, P, KT, P], BF16, kind="ExternalInput").ap()
    wqT = nc.dram_tensor("wqT", [P, KT, DG], BF16, kind="ExternalInput").ap()
    wkT = nc.dram_tensor("wkT", [P, KT, DG], BF16, kind="ExternalInput").ap()
    wvT = nc.dram_tensor("wvT", [P, KT, DG], BF16, kind="ExternalInput").ap()
    woT = nc.dram_tensor("woT", [P, 2, D_MODEL], BF16, kind="ExternalInput").ap()
    bq_d = nc.dram_tensor("bq_s", [DG], F32, kind="ExternalInput").ap()
    yT_d = nc.dram_tensor("yT", [D_MODEL, N], F32, kind="ExternalOutput").ap()
    if DEBUG_DUMP:
        dbg_kt = nc.dram_tensor("dbg_kt", [P, 2, N], F32, kind="ExternalOutput").ap()
        dbg_v = nc.dram_tensor("dbg_v", [P, NKT, HG, DK + 1], F32,
                               kind="ExternalOutput").ap()
        dbg_qt = nc.dram_tensor("dbg_qt", [P, 2, QB], F32, kind="ExternalOutput").ap()
        dbg_o = nc.dram_tensor("dbg_o", [P, 2, QB], F32, kind="ExternalOutput").ap()
        dbg_at = nc.dram_tensor("dbg_at", [P, 2, QB], F32, kind="ExternalOutput").ap()
        dbg_av = nc.dram_tensor("dbg_av", [P, QB], F32, kind="ExternalOutput").ap()
        dbg_bc = nc.dram_tensor("dbg_bc", [P, QB], F32, kind="ExternalOutput").ap()

    bq_v = bq_d.rearrange("(j p) -> p j", p=P)        # [128, 2]

    with tile.TileContext(nc) as tc:
        with ExitStack() as ctx:
            const = ctx.enter_context(tc.tile_pool(name="const", bufs=1))
            xin = ctx.enter_context(tc.tile_pool(name="xin", bufs=2))
            xvp = ctx.enter_context(tc.tile_pool(name="xvp", bufs=3))
            atp = ctx.enter_context(tc.tile_pool(name="atp", bufs=3))
            smal = ctx.enter_context(tc.tile_pool(name="smal", bufs=2))
            stp = ctx.enter_context(tc.tile_pool(name="stp", bufs=2, space="PSUM"))
            avp = ctx.enter_context(tc.tile_pool(name="avp", bufs=2, space="PSUM"))
            aux = ctx.enter_context(tc.tile_pool(name="aux", bufs=2, space="PSUM"))

            # ---- constants: weights on the ACT queue (idle at start),
            # x chunks on SP, xv on DVE ----
            wk_sb = const.tile([P, KT, DG], BF16, tag="wk")
            wv_sb = const.tile([P, KT, DG], BF16, tag="wv")
            wq_sb = const.tile([P, KT, DG], BF16, tag="wq")
            wo_sb = const.tile([P, 2, D_MODEL], BF16, tag="wo")
            nc.sync.dma_start(wk_sb[:], wkT)  # first: gates the K projections
            nc.scalar.dma_start(wv_sb[:], wvT)
            nc.scalar.dma_start(wq_sb[:], wqT)
            nc.scalar.dma_start(wo_sb[:], woT)
            bq_sb = const.tile([P, 2], F32, tag="bq")
            nc.scalar.dma_start(bq_sb[:], bq_v)

            # K^T stored zero-padded per head-half: kt2_sb[:, hp, pair, n]
            # has K_h dims on partitions hp*64..hp*64+64 and ZEROS on the
            # other 64 partitions, so S matmuls contract over the full 128
            # partitions in (128,128) array mode — no tiling-mode switches
            # (each 64<->128 mode change drains the tensor engine).
            kt2_sb = const.tile([P, 2, 2, N], BF16, tag="kt2")
            nc.vector.memset(kt2_sb.rearrange("p a b c -> p (a b c)"), 0.0)
            v_sb = const.tile([P, NKT, HG, DK + 1], BF16, tag="v")  # [V_h | 1]
            # whole-tile memset to 1.0; V-proj copies overwrite cols 0:64,
            # leaving the denominator ones-column.  (A strided ones DMA here
            # generates thousands of 2-byte descriptors and stalls the DMA
            # engines for ~50us.)
            nc.vector.memset(
                v_sb.rearrange("p a b c -> p (a b c)"), 1.0)

            qts = {}
            o_sb = {}
            for c in range(NQB):
                qts[c] = const.tile([P, 2, QB], BF16, tag=f"qt{c}", name=f"qt_{c}")
                o_sb[c] = const.tile([P, 2, QB], BF16, tag=f"o{c}", name=f"o_{c}")

            # x DMAs up front; xk0 + xq0 first so the head (K0 + Q0
            # projections) is never queued behind the xv stream
            xks, xqs, xvs = {}, {}, {}

            def load_xk(c):
                xks[c] = xin.tile([P, KT, QB], BF16, tag="xk", name=f"xk_{c}",
                                  bufs=4)
                nc.sync.dma_start(xks[c][:], xkT[c])

            def load_xq(c):
                xqs[c] = xin.tile([P, KT, QB], BF16, tag="xq", name=f"xq_{c}",
                                  bufs=4)
                nc.sync.dma_start(xqs[c][:], xqT[c])

            for c in range(NQB):
                load_xk(c)
            load_xq(0)
            for nt in range(NKT):
                xvs[nt] = xvp.tile([P, KT, P], BF16, tag="xv", name=f"xv_{nt}",
                                   bufs=8)
                nc.sync.dma_start(xvs[nt][:], xvT[nt])
            for c in range(1, NQB):
                load_xq(c)

            # ---------------- emission helpers ----------------
            def k_proj(c, j):
                ps = aux.tile([P, QB], F32, tag="aux", name=f"kps_{c}_{j}")
                for k in range(KT):
                    nc.tensor.matmul(
                        ps[:], wk_sb[:, k, j * P:(j + 1) * P], xks[c][:, k, :],
                        start=(k == 0), stop=(k == KT - 1))
                sl = slice(c * QB, (c + 1) * QB)
                nc.vector.tensor_copy(kt2_sb[0:DK, 0, j, sl], ps[0:DK, :])
                nc.vector.tensor_copy(kt2_sb[DK:P, 1, j, sl], ps[DK:P, :])

            def q_proj(c, j):
                ps = aux.tile([P, QB], F32, tag="aux", name=f"qps_{c}_{j}")
                for k in range(KT):
                    nc.tensor.matmul(
                        ps[:], wq_sb[:, k, j * P:(j + 1) * P], xqs[c][:, k, :],
                        start=(k == 0), stop=(k == KT - 1))
                nc.vector.tensor_tensor(
                    qts[c][:, j, :], ps[:],
                    bq_sb[:, j, None].to_broadcast((P, QB)), ADD)

            vps_live = {}

            def v_proj_half(nt, half):
                if half == 0:
                    vps_live[nt] = aux.tile([P, QB], F32, tag="aux",
                                            name=f"vps_{nt}")
                ps = vps_live[nt]
                for k in range(half * 4, half * 4 + 4):
                    nc.tensor.matmul(ps[:, 0:DG], xvs[nt][:, k, :], wv_sb[:, k, :],
                                     start=(k == 0), stop=(k == KT - 1))
                if half == 1:
                    nc.vector.tensor_copy(
                        v_sb[:, nt, :, 0:DK],
                        ps[:, 0:DG].rearrange("p (h d) -> p h d", h=HG))
                    del vps_live[nt]

            def o_proj(c, m):
                yps = aux.tile([P, QB], F32, tag="aux", name=f"yps_{c}_{m}")
                for j in range(2):
                    nc.tensor.matmul(
                        yps[:], wo_sb[:, j, m * P:(m + 1) * P], o_sb[c][:, j, :],
                        start=(j == 0), stop=(j == 1))
                y_sb = smal.tile([P, QB], F32, tag="y", name=f"y_{c}_{m}",
                                 bufs=8)
                if c == NQB - 1:
                    # tail only: ACT is idle there, DVE runs the normalize
                    nc.scalar.copy(y_sb[:], yps[:])
                else:
                    nc.vector.tensor_copy(y_sb[:], yps[:])
                # all output DMAs on the (idle) SP queue: SWDGE generation on
                # the gpsimd queue jams the Pool engine ahead of the softmax
                # broadcasts and stalls the tail ~9us
                nc.sync.dma_start(
                    yT_d[m * P:(m + 1) * P, c * QB:(c + 1) * QB], y_sb[:])

            # filler queue: PE work units (projections, out-projections)
            # interleaved into the exp-paced attention loop.  Forced drains
            # keep dependencies ahead: K chunk kt//4 before S(kt), V tile kt
            # before AV(kt), Q chunk c before attention(c).
            state = {"kj0": 1, "kj1": 0, "q1": 0, "v": 0, "q": 1}
            filler = []
            # weave: V halves (needed by AV(kt)) interleaved with the
            # deferred K j0 chunks (needed by S(kt) of pair 0) and, later,
            # the j1 halves (needed only from pair 1 of chunk 0 onward)
            for nt in range(NKT):
                filler.append(("v", nt, lambda nt=nt: v_proj_half(nt, 0)))
                filler.append(("v", nt + 1, lambda nt=nt: v_proj_half(nt, 1)))
                if nt < 3:
                    filler.append(("kj0", nt + 2, lambda c=nt + 1: k_proj(c, 0)))
                elif nt < 7:
                    filler.append(("kj1", nt - 2, lambda c=nt - 3: k_proj(c, 1)))
                elif nt == 7:
                    filler.append(("q1", 1, lambda: q_proj(0, 1)))
            for c in range(1, NQB):
                filler.append(("q", c, lambda c=c: q_proj(c, 0)))
                filler.append(("q", c + 1, lambda c=c: q_proj(c, 1)))

            def pop_filler(k=1):
                for _ in range(k):
                    if filler:
                        kind, idx, fn = filler.pop(0)
                        fn()
                        if kind in state:
                            state[kind] = max(state[kind], idx)

            def drain_until(kind, idx):
                while state.get(kind, idx) < idx and filler:
                    pop_filler()

            def emit_av(c, pair, avs, ats, kt):
                drain_until("v", kt + 1)
                for hp in range(2):
                    h = 2 * pair + hp
                    nc.tensor.matmul(
                        avs[hp][:], v_sb[:, kt, h, :], ats[kt][:, hp, :],
                        start=(kt == 0), stop=(kt == NKT - 1))

            # ---- head: only what the very first exp needs (K chunk0 j=0,
            # Q chunk0 j=0); everything else drains as woven filler ----
            k_proj(0, 0)
            q_proj(0, 0)

            # ---- attention ----
            dbg_at_sb = {}

            def close_pair(c, pair, avs, ats):
                """Last AV + accumulator evacuation + softmax normalization
                for (c, pair).  Deferred into the NEXT pair's loop so the
                boundary never stalls the exp stream."""
                emit_av(c, pair, avs, ats, NKT - 1)
                if DEBUG_DUMP and c == 0 and pair == 0:
                    snap_av = const.tile([P, QB], F32, tag="dbgav")
                    nc.vector.tensor_copy(snap_av[0:DK + 1, :],
                                          avs[0][0:DK + 1, :])
                    dbg_at_sb["av"] = snap_av
                # evacuate the accumulators to SBUF immediately: one DVE
                # copy frees the PSUM slot so the next pair's AV matmuls
                # don't wait for the (slow) normalization chain.  The very
                # last pair skips evacuation (no successor) and multiplies
                # straight from PSUM off the tail's critical path.
                last = (c == NQB - 1 and pair == 1)
                avsb = {}
                zrs = {}
                for hp in range(2):
                    zrs[hp] = smal.tile([1, QB], F32, tag=f"zr{hp}",
                                        name=f"zr_{c}_{pair}_{hp}")
                    nc.vector.tensor_copy(zrs[hp][:], avs[hp][DK:DK + 1, :])
                    if last:
                        avsb[hp] = avs[hp][0:DK, :]
                    else:
                        avsb[hp] = smal.tile([DK, QB], F32, tag=f"avsb{hp}",
                                             name=f"avsb_{c}_{pair}_{hp}")
                        nc.vector.tensor_copy(avsb[hp][:], avs[hp][0:DK, :])
                zbcs = {}
                for hp in range(2):
                    rzf = smal.tile([1, QB], F32, tag="rzf",
                                    name=f"rzf_{c}_{pair}_{hp}")
                    nc.vector.reciprocal_approx_fast(rzf[:], zrs[hp][:])
                    zbcs[hp] = smal.tile([DK, QB], F32, tag=f"zbc{hp}",
                                         name=f"zbc_{c}_{pair}_{hp}")
                    nc.gpsimd.partition_broadcast(zbcs[hp][:], rzf[:],
                                                  channels=DK)
                for hp in range(2):
                    if DEBUG_DUMP and c == 0 and pair == 0 and hp == 0:
                        snap_bc = const.tile([P, QB], F32, tag="dbgbc")
                        nc.vector.tensor_copy(snap_bc[0:DK, :], zbcs[0][:])
                        dbg_at_sb["bc"] = snap_bc
                    nc.vector.tensor_tensor(
                        o_sb[c][DK * hp:DK * (hp + 1), pair, :],
                        avsb[hp][:], zbcs[hp][:], MULT)
                if pair == 1:
                    # chunk done: its out-projection becomes filler
                    for m in range(8):
                        filler.append(("o", 0, lambda c=c, m=m: o_proj(c, m)))

            # flat step schedule; each step emits the NEXT step's S matmuls
            # first so the S stream leads the PE queue and the exp gate for
            # step i+1 is already satisfied when exp(i) finishes
            steps = [(c, pair, kt)
                     for c in range(NQB) for pair in range(2)
                     for kt in range(NKT)]
            stbs = {}

            def emit_s(i):
                c, pair, kt = steps[i]
                if pair == 0 and kt == 0:
                    drain_until("q", c + 1)
                if pair == 1 and kt == 0:
                    drain_until("q1", 1)
                drain_until("kj0" if pair == 0 else "kj1", kt // 4 + 1)
                stb = stp.tile([P, 2, QB], F32, tag="st",
                               name=f"st_{c}_{pair}_{kt}")
                for hp in range(2):
                    nc.tensor.matmul(
                        stb[:, hp, :],
                        kt2_sb[:, hp, pair, kt * P:(kt + 1) * P],
                        qts[c][:, pair, :],
                        start=True, stop=True)
                stbs[i] = stb

            pending_close = None
            avs = None
            ats = {}
            emit_s(0)
            for i, (c, pair, kt) in enumerate(steps):
                if kt == 0:
                    avs = [avp.tile([DK + 1, QB], F32, tag="av",
                                    name=f"av_{c}_{pair}_{hp}")
                           for hp in range(2)]
                    ats = {}
                if i + 1 < len(steps):
                    emit_s(i + 1)
                at = atp.tile([P, 2, QB], BF16, tag="at",
                              name=f"at_{c}_{pair}_{kt}")
                nc.scalar.activation(
                    at.rearrange("p a b -> p (a b)"),
                    stbs.pop(i).rearrange("p a b -> p (a b)"), EXP, scale=0.125)
                ats[kt] = at
                if DEBUG_DUMP and c == 0 and pair == 0 and kt == 0:
                    snap = const.tile([P, 2, QB], F32, tag="dbgsnap")
                    nc.vector.tensor_copy(
                        snap.rearrange("p a b -> p (a b)"),
                        at.rearrange("p a b -> p (a b)"))
                    dbg_at_sb[0] = snap
                if kt == 0 and pending_close is not None:
                    pending_close()
                    pending_close = None
                # AV lags one k-tile so exp latency is hidden
                if kt > 0:
                    emit_av(c, pair, avs, ats, kt - 1)
                pop_filler()
                if kt == NKT - 1:
                    pending_close = (lambda c=c, pair=pair, avs=avs, ats=ats:
                                     close_pair(c, pair, avs, ats))
            pending_close()

            # drain remaining filler (last chunk's out-projection)
            pop_filler(len(filler))

            if DEBUG_DUMP:
                for nm, sb, dr in (("kt", kt_sb, dbg_kt), ("v", v_sb, dbg_v),
                                   ("qt", qts[0], dbg_qt), ("o", o_sb[0], dbg_o),
                                   ("at", dbg_at_sb[0], dbg_at)):
                    f = smal.tile(list(sb.shape), F32, tag=f"dbg{nm}",
                                  name=f"dbgt_{nm}")
                    nc.vector.tensor_copy(
                        f.rearrange("p a b c -> p (a b c)") if len(sb.shape) == 4
                        else f.rearrange("p a b -> p (a b)"),
                        sb.rearrange("p a b c -> p (a b c)") if len(sb.shape) == 4
                        else sb.rearrange("p a b -> p (a b)"))
                    nc.sync.dma_start(dr, f[:])
                nc.sync.dma_start(dbg_av, dbg_at_sb["av"][:])
                nc.sync.dma_start(dbg_bc, dbg_at_sb["bc"][:])

    nc.compile()
    return nc


def get_program():
    global _PROGRAM
    if _PROGRAM is None:
        _PROGRAM = _build_program()
    return _PROGRAM


def _tile_xT(x, nblk, width):
    # x [n, 1024] f32 -> x^T bf16 tiled [nblk, 128 p, 8 k, width]
    xt = np.ascontiguousarray(x.T).astype(NP_BF16)       # [1024, n]
    return np.ascontiguousarray(
        xt.reshape(KT, P, nblk, width).transpose(2, 1, 0, 3))


def _tile_w(w_rows):
    # w_rows [256, 1024] (= W[g-slice]) -> W^T bf16 tiled [128 p, 8 k, 256]
    return np.ascontiguousarray(
        w_rows.T.astype(NP_BF16).reshape(KT, P, DG).transpose(1, 0, 2))


def make_in_maps(queries, keys, values, Wq, bq, Wk, bk, Wv, bv, Wo, bo):
    """Build per-core input dicts. Core c handles batch c//4, head group c%4."""
    f32 = np.float32
    xT = {}
    for ib in range(B):
        xT[ib] = (
            _tile_xT(np.asarray(queries[ib], f32), NQB, QB),
            _tile_xT(np.asarray(keys[ib], f32), NQB, QB),
            _tile_xT(np.asarray(values[ib], f32), NKT, P),
        )
    in_maps = []
    for core in range(8):
        ib, g = core // G, core % G
        sl = slice(g * DG, (g + 1) * DG)
        in_maps.append({
            "xqT": xT[ib][0], "xkT": xT[ib][1], "xvT": xT[ib][2],
            "wqT": _tile_w(Wq[sl, :]),
            "wkT": _tile_w(Wk[sl, :]),
            "wvT": _tile_w(Wv[sl, :]),
            "woT": np.ascontiguousarray(
                Wo[:, sl].T.astype(NP_BF16).reshape(2, P, D_MODEL)
                .transpose(1, 0, 2)),
            "bq_s": np.ascontiguousarray(bq[sl]).astype(f32),
        })
    return in_maps


def gather_output(results, Wo, bv, bo):
    out = np.zeros((B, N, D_MODEL), np.float32)
    for core in range(8):
        out[core // G] += results[core]["yT"].T
    # K bias is softmax-invariant (per-query constant logit shift); V bias
    # commutes through normalized attention into a constant output offset.
    const = bo.astype(np.float64) + Wo.astype(np.float64) @ bv.astype(np.float64)
    out += const[None, None, :].astype(np.float32)
    return out


def _run(inputs, trace=False, **spmd_kwargs):
    nc = get_program()
    in_maps = make_in_maps(**inputs)
    res = run_bass_kernel_spmd(nc, in_maps, core_ids=list(range(8)),
                               trace=trace, **spmd_kwargs)
    return gather_output(res.results, inputs["Wo"], inputs["bv"],
                         inputs["bo"]), res


def kernel(**inputs) -> np.ndarray:
    inputs = {k: np.asarray(v, dtype=np.float32) for k, v in inputs.items()}
    out, _ = _run(inputs, trace=False)
    return out


# revision 67
# speedup vs baseline: 1.1492x; 1.1492x over previous
"""Multi-head attention (b=2, n=2048, d_model=1024, H=16, d_k=d_v=64) on 8
Trainium2 NeuronCores.

Sharding: 8 cores = 2 (batch) x 4 (head groups of 4 heads).  Each core
computes, for its batch ib and head group g (heads as 2 pairs x 2 hp):

    qT/kT projections   qT = Wq_g @ x^T            [256, 2048]
    v projection        V  = x @ Wv_g^T            [2048, 256]
    per head: S^T = K_h Q_h^T (kpos on partitions), A^T = exp(S^T/8),
              O^T|Z = [V_h|1]^T A^T  (PSUM row 64 gives softmax denom Z)
    normalize O^T by 1/Z (Z broadcast across partitions via a tiny
    ones-outer-product matmul on the PE -- no DRAM roundtrip),
    out-projection Y^T = Wo_g @ O_cat^T            [1024, 2048]

Host sums the 4 per-group partial Y^T per batch and adds bo + Wo@bv
(the V bias commutes through normalized attention; the K bias is a
per-query constant shift of the logits, to which softmax is exactly
invariant, so both are folded out of the device program).

All matmul operands are bf16 (PSUM accumulation stays fp32): this halves
input DMA, enables Fast Weight Load on the PE (2x faster LDWEIGHTS), and
doubles DVE throughput where it applies.  exp is evaluated on fp32 PSUM
logits.  Softmax skips max-subtraction: scores*scale are ~N(0,1).

The PE instruction stream is software-pipelined: within the attention
inner loop the AV matmuls lag the S matmuls by one k-tile so the ACT
(exp) stream is hidden, and projection / out-projection matmuls are
sprinkled between attention tiles as filler so the PE never idles long
enough for the HAM clock gate to re-throttle.
"""

import numpy as np
from contextlib import ExitStack

import ml_dtypes

import concourse.bass as bass
import concourse.mybir as mybir
import concourse.tile as tile
from concourse import bacc
from concourse.bass_utils import run_bass_kernel_spmd

F32 = mybir.dt.float32
BF16 = mybir.dt.bfloat16
NP_BF16 = ml_dtypes.bfloat16
EXP = mybir.ActivationFunctionType.Exp
ADD = mybir.AluOpType.add
MULT = mybir.AluOpType.mult

D_MODEL = 1024
H = 16
DK = 64
B = 2
N = 2048           # nq = nk
G = 4              # head groups (cores per batch)
HG = H // G        # heads per group = 4
DG = HG * DK       # 256 group dims
KT = 8             # D_MODEL / 128 contraction tiles
NKT = N // 128     # 16 k-tiles in attention
QB = 512           # attention q-block
NQB = N // QB      # 4 q-blocks
P = 128

_PROGRAM = None
DEBUG_DUMP = False


def _build_program():
    nc = bacc.Bacc("TRN2", target_bir_lowering=False, debug=False, num_devices=8)

    # pre-tiled on host (bf16): per-partition lines are contiguous reads
    xqT = nc.dram_tensor("xqT", [NQB, P, KT, QB], BF16, kind="ExternalInput").ap()
    xkT = nc.dram_tensor("xkT", [NQB, P, KT, QB], BF16, kind="ExternalInput").ap()
    xvT = nc.dram_tensor("xvT", [NKT, P, KT, P], BF16, kind="ExternalInput").ap()
    wqT = nc.dram_tensor("wqT", [P, KT, DG], BF16, kind="ExternalInput").ap()
    wkT = nc.dram_tensor("wkT", [P, KT, DG], BF16, kind="ExternalInput").ap()
    wvT = nc.dram_tensor("wvT", [P, KT, DG], BF16, kind="ExternalInput").ap()
    woT = nc.dram_tensor("woT", [P, 2, D_MODEL], BF16, kind="ExternalInput").ap()
    bq_d = nc.dram_tensor("bq_s", [DG], F32, kind="ExternalInput").ap()
    yT_d = nc.dram_tensor("yT", [D_MODEL, N], F32, kind="ExternalOutput").ap()
    if DEBUG_DUMP:
        dbg_kt = nc.dram_tensor("dbg_kt", [P, 2, N], F32, kind="ExternalOutput").ap()
        dbg_v = nc.dram_tensor("dbg_v", [P, NKT, HG, DK + 1], F32,
                               kind="ExternalOutput").ap()
        dbg_qt = nc.dram_tensor("dbg_qt", [P, 2, QB], F32, kind="ExternalOutput").ap()
        dbg_o = nc.dram_tensor("dbg_o", [P, 2, QB], F32, kind="ExternalOutput").ap()
        dbg_at = nc.dram_tensor("dbg_at", [P, 2, QB], F32, kind="ExternalOutput").ap()
        dbg_av = nc.dram_tensor("dbg_av", [P, QB], F32, kind="ExternalOutput").ap()
        dbg_bc = nc.dram_tensor("dbg_bc", [P, QB], F32, kind="ExternalOutput").ap()

    bq_v = bq_d.rearrange("(j p) -> p j", p=P)        # [128, 2]

    with tile.TileContext(nc) as tc:
        with ExitStack() as ctx:
            const = ctx.enter_context(tc.tile_pool(name="const", bufs=1))
            xin = ctx.enter_context(tc.tile_pool(name="xin", bufs=2))
            xvp = ctx.enter_context(tc.tile_pool(name="xvp", bufs=3))
            atp = ctx.enter_context(tc.tile_pool(name="atp", bufs=4))
            smal = ctx.enter_context(tc.tile_pool(name="smal", bufs=2))
            stp = ctx.enter_context(tc.tile_pool(name="stp", bufs=2, space="PSUM"))
            avp = ctx.enter_context(tc.tile_pool(name="avp", bufs=2, space="PSUM"))
            aux = ctx.enter_context(tc.tile_pool(name="aux", bufs=2, space="PSUM"))

            # ---- constants: weights on the ACT queue (idle at start),
            # x chunks on SP, xv on DVE ----
            wk_sb = const.tile([P, KT, DG], BF16, tag="wk")
            wv_sb = const.tile([P, KT, DG], BF16, tag="wv")
            wq_sb = const.tile([P, KT, DG], BF16, tag="wq")
            wo_sb = const.tile([P, 2, D_MODEL], BF16, tag="wo")
            nc.sync.dma_start(wk_sb[:], wkT)  # first: gates the K projections
            nc.scalar.dma_start(wv_sb[:], wvT)
            nc.scalar.dma_start(wq_sb[:], wqT)
            nc.scalar.dma_start(wo_sb[:], woT)
            bq_sb = const.tile([P, 2], F32, tag="bq")
            nc.scalar.dma_start(bq_sb[:], bq_v)

            # K^T stored zero-padded per head-half: kt2_sb[:, hp, pair, n]
            # has K_h dims on partitions hp*64..hp*64+64 and ZEROS on the
            # other 64 partitions, so S matmuls contract over the full 128
            # partitions in (128,128) array mode — no tiling-mode switches
            # (each 64<->128 mode change drains the tensor engine).
            kt2_sb = const.tile([P, 2, 2, N], BF16, tag="kt2")
            nc.vector.memset(kt2_sb.rearrange("p a b c -> p (a b c)"), 0.0)
            v_sb = const.tile([P, NKT, HG, DK + 1], BF16, tag="v")  # [V_h | 1]
            # whole-tile memset to 1.0; V-proj copies overwrite cols 0:64,
            # leaving the denominator ones-column.  (A strided ones DMA here
            # generates thousands of 2-byte descriptors and stalls the DMA
            # engines for ~50us.)
            nc.vector.memset(
                v_sb.rearrange("p a b c -> p (a b c)"), 1.0)

            qts = {}
            o_sb = {}
            for c in range(NQB):
                qts[c] = const.tile([P, 2, QB], BF16, tag=f"qt{c}", name=f"qt_{c}")
                o_sb[c] = const.tile([P, 2, QB], BF16, tag=f"o{c}", name=f"o_{c}")

            # x DMAs up front; xk0 + xq0 first so the head (K0 + Q0
            # projections) is never queued behind the xv stream
            xks, xqs, xvs = {}, {}, {}

            def load_xk(c):
                xks[c] = xin.tile([P, KT, QB], BF16, tag="xk", name=f"xk_{c}",
                                  bufs=4)
                nc.sync.dma_start(xks[c][:], xkT[c])

            def load_xq(c):
                xqs[c] = xin.tile([P, KT, QB], BF16, tag="xq", name=f"xq_{c}",
                                  bufs=4)
                nc.sync.dma_start(xqs[c][:], xqT[c])

            for c in range(NQB):
                load_xk(c)
            load_xq(0)
            for nt in range(NKT):
                xvs[nt] = xvp.tile([P, KT, P], BF16, tag="xv", name=f"xv_{nt}",
                                   bufs=8)
                nc.sync.dma_start(xvs[nt][:], xvT[nt])
            for c in range(1, NQB):
                load_xq(c)

            # ---------------- emission helpers ----------------
            def k_proj(c, j):
                ps = aux.tile([P, QB], F32, tag="aux", name=f"kps_{c}_{j}")
                for k in range(KT):
                    nc.tensor.matmul(
                        ps[:], wk_sb[:, k, j * P:(j + 1) * P], xks[c][:, k, :],
                        start=(k == 0), stop=(k == KT - 1))
                sl = slice(c * QB, (c + 1) * QB)
                nc.vector.tensor_copy(kt2_sb[0:DK, 0, j, sl], ps[0:DK, :])
                nc.vector.tensor_copy(kt2_sb[DK:P, 1, j, sl], ps[DK:P, :])

            def q_proj(c, j):
                ps = aux.tile([P, QB], F32, tag="aux", name=f"qps_{c}_{j}")
                for k in range(KT):
                    nc.tensor.matmul(
                        ps[:], wq_sb[:, k, j * P:(j + 1) * P], xqs[c][:, k, :],
                        start=(k == 0), stop=(k == KT - 1))
                nc.vector.tensor_tensor(
                    qts[c][:, j, :], ps[:],
                    bq_sb[:, j, None].to_broadcast((P, QB)), ADD)

            vps_live = {}

            def v_proj_half(nt, half):
                if half == 0:
                    vps_live[nt] = aux.tile([P, QB], F32, tag="aux",
                                            name=f"vps_{nt}")
                ps = vps_live[nt]
                for k in range(half * 4, half * 4 + 4):
                    nc.tensor.matmul(ps[:, 0:DG], xvs[nt][:, k, :], wv_sb[:, k, :],
                                     start=(k == 0), stop=(k == KT - 1))
                if half == 1:
                    nc.vector.tensor_copy(
                        v_sb[:, nt, :, 0:DK],
                        ps[:, 0:DG].rearrange("p (h d) -> p h d", h=HG))
                    del vps_live[nt]

            def o_proj(c, m):
                yps = aux.tile([P, QB], F32, tag="aux", name=f"yps_{c}_{m}")
                for j in range(2):
                    nc.tensor.matmul(
                        yps[:], wo_sb[:, j, m * P:(m + 1) * P], o_sb[c][:, j, :],
                        start=(j == 0), stop=(j == 1))
                y_sb = smal.tile([P, QB], F32, tag="y", name=f"y_{c}_{m}",
                                 bufs=8)
                if c == NQB - 1:
                    # tail only: ACT is idle there, DVE runs the normalize
                    nc.scalar.copy(y_sb[:], yps[:])
                else:
                    nc.vector.tensor_copy(y_sb[:], yps[:])
                # all output DMAs on the (idle) SP queue: SWDGE generation on
                # the gpsimd queue jams the Pool engine ahead of the softmax
                # broadcasts and stalls the tail ~9us
                nc.sync.dma_start(
                    yT_d[m * P:(m + 1) * P, c * QB:(c + 1) * QB], y_sb[:])

            # filler queue: PE work units (projections, out-projections)
            # interleaved into the exp-paced attention loop.  Forced drains
            # keep dependencies ahead: K chunk kt//4 before S(kt), V tile kt
            # before AV(kt), Q chunk c before attention(c).
            state = {"k": NQB, "v": 0, "q": 1}
            filler = []
            for nt in range(NKT):
                filler.append(("v", nt, lambda nt=nt: v_proj_half(nt, 0)))
                filler.append(("v", nt + 1, lambda nt=nt: v_proj_half(nt, 1)))
            for c in range(1, NQB):
                filler.append(("q", c, lambda c=c: q_proj(c, 0)))
                filler.append(("q", c + 1, lambda c=c: q_proj(c, 1)))

            def pop_filler(k=1):
                for _ in range(k):
                    if filler:
                        kind, idx, fn = filler.pop(0)
                        fn()
                        if kind in state:
                            state[kind] = max(state[kind], idx)

            def drain_until(kind, idx):
                while state.get(kind, idx) < idx and filler:
                    pop_filler()

            def emit_av(c, pair, avs, ats, kt):
                drain_until("v", kt + 1)
                for hp in range(2):
                    h = 2 * pair + hp
                    nc.tensor.matmul(
                        avs[hp][:], v_sb[:, kt, h, :], ats[kt][:, hp, :],
                        start=(kt == 0), stop=(kt == NKT - 1))

            # ---- head: all K chunks + Q chunk 0; V and later Q chunks
            # drain as filler inside the exp-paced attention loop ----
            for c4 in range(NQB):
                k_proj(c4, 0)
                k_proj(c4, 1)
            q_proj(0, 0)
            q_proj(0, 1)

            # ---- attention ----
            dbg_at_sb = {}

            def close_pair(c, pair, avs, ats):
                """Last AV + accumulator evacuation + softmax normalization
                for (c, pair).  Deferred into the NEXT pair's loop so the
                boundary never stalls the exp stream."""
                emit_av(c, pair, avs, ats, NKT - 1)
                if DEBUG_DUMP and c == 0 and pair == 0:
                    snap_av = const.tile([P, QB], F32, tag="dbgav")
                    nc.vector.tensor_copy(snap_av[0:DK + 1, :],
                                          avs[0][0:DK + 1, :])
                    dbg_at_sb["av"] = snap_av
                # evacuate the accumulators to SBUF immediately: one DVE
                # copy frees the PSUM slot so the next pair's AV matmuls
                # don't wait for the (slow) normalization chain.  The very
                # last pair skips evacuation (no successor) and multiplies
                # straight from PSUM off the tail's critical path.
                last = (c == NQB - 1 and pair == 1)
                avsb = {}
                zrs = {}
                for hp in range(2):
                    zrs[hp] = smal.tile([1, QB], F32, tag=f"zr{hp}",
                                        name=f"zr_{c}_{pair}_{hp}")
                    nc.vector.tensor_copy(zrs[hp][:], avs[hp][DK:DK + 1, :])
                    if last:
                        avsb[hp] = avs[hp][0:DK, :]
                    else:
                        avsb[hp] = smal.tile([DK, QB], F32, tag=f"avsb{hp}",
                                             name=f"avsb_{c}_{pair}_{hp}")
                        nc.vector.tensor_copy(avsb[hp][:], avs[hp][0:DK, :])
                zbcs = {}
                for hp in range(2):
                    rzf = smal.tile([1, QB], F32, tag="rzf",
                                    name=f"rzf_{c}_{pair}_{hp}")
                    nc.vector.reciprocal_approx_fast(rzf[:], zrs[hp][:])
                    zbcs[hp] = smal.tile([DK, QB], F32, tag=f"zbc{hp}",
                                         name=f"zbc_{c}_{pair}_{hp}")
                    nc.gpsimd.partition_broadcast(zbcs[hp][:], rzf[:],
                                                  channels=DK)
                for hp in range(2):
                    if DEBUG_DUMP and c == 0 and pair == 0 and hp == 0:
                        snap_bc = const.tile([P, QB], F32, tag="dbgbc")
                        nc.vector.tensor_copy(snap_bc[0:DK, :], zbcs[0][:])
                        dbg_at_sb["bc"] = snap_bc
                    nc.vector.tensor_tensor(
                        o_sb[c][DK * hp:DK * (hp + 1), pair, :],
                        avsb[hp][:], zbcs[hp][:], MULT)
                if pair == 1:
                    # chunk done: its out-projection becomes filler
                    for m in range(8):
                        filler.append(("o", 0, lambda c=c, m=m: o_proj(c, m)))

            # flat step schedule; each step emits the NEXT step's S matmuls
            # first so the S stream leads the PE queue and the exp gate for
            # step i+1 is already satisfied when exp(i) finishes
            steps = [(c, pair, kt)
                     for c in range(NQB) for pair in range(2)
                     for kt in range(NKT)]
            stbs = {}

            def emit_s(i):
                c, pair, kt = steps[i]
                if pair == 0 and kt == 0:
                    drain_until("q", c + 1)
                stb = stp.tile([P, 2, QB], F32, tag="st",
                               name=f"st_{c}_{pair}_{kt}")
                for hp in range(2):
                    nc.tensor.matmul(
                        stb[:, hp, :],
                        kt2_sb[:, hp, pair, kt * P:(kt + 1) * P],
                        qts[c][:, pair, :],
                        start=True, stop=True)
                stbs[i] = stb

            pending_close = None
            avs = None
            ats = {}
            emit_s(0)
            for i, (c, pair, kt) in enumerate(steps):
                if kt == 0:
                    avs = [avp.tile([DK + 1, QB], F32, tag="av",
                                    name=f"av_{c}_{pair}_{hp}")
                           for hp in range(2)]
                    ats = {}
                if i + 1 < len(steps):
                    emit_s(i + 1)
                at = atp.tile([P, 2, QB], BF16, tag="at",
                              name=f"at_{c}_{pair}_{kt}")
                nc.scalar.activation(
                    at.rearrange("p a b -> p (a b)"),
                    stbs.pop(i).rearrange("p a b -> p (a b)"), EXP, scale=0.125)
                ats[kt] = at
                if DEBUG_DUMP and c == 0 and pair == 0 and kt == 0:
                    snap = const.tile([P, 2, QB], F32, tag="dbgsnap")
                    nc.vector.tensor_copy(
                        snap.rearrange("p a b -> p (a b)"),
                        at.rearrange("p a b -> p (a b)"))
                    dbg_at_sb[0] = snap
                if kt == 0 and pending_close is not None:
                    pending_close()
                    pending_close = None
                # AV lags one k-tile so exp latency is hidden
                if kt > 0:
                    emit_av(c, pair, avs, ats, kt - 1)
                # rate-limit cadence pops: exp-stream slips are permanent
                # (the exp period cannot compress), so fillers must be
                # spaced against the ~200ns/kt PE lead rather than popped
                # greedily in chunk-start bursts
                if kt % 3 == 2:
                    pop_filler()
                if kt == NKT - 1:
                    pending_close = (lambda c=c, pair=pair, avs=avs, ats=ats:
                                     close_pair(c, pair, avs, ats))
            pending_close()

            # drain remaining filler (last chunk's out-projection)
            pop_filler(len(filler))

            if DEBUG_DUMP:
                for nm, sb, dr in (("kt", kt_sb, dbg_kt), ("v", v_sb, dbg_v),
                                   ("qt", qts[0], dbg_qt), ("o", o_sb[0], dbg_o),
                                   ("at", dbg_at_sb[0], dbg_at)):
                    f = smal.tile(list(sb.shape), F32, tag=f"dbg{nm}",
                                  name=f"dbgt_{nm}")
                    nc.vector.tensor_copy(
                        f.rearrange("p a b c -> p (a b c)") if len(sb.shape) == 4
                        else f.rearrange("p a b -> p (a b)"),
                        sb.rearrange("p a b c -> p (a b c)") if len(sb.shape) == 4
                        else sb.rearrange("p a b -> p (a b)"))
                    nc.sync.dma_start(dr, f[:])
                nc.sync.dma_start(dbg_av, dbg_at_sb["av"][:])
                nc.sync.dma_start(dbg_bc, dbg_at_sb["bc"][:])

    nc.compile()
    return nc


def get_program():
    global _PROGRAM
    if _PROGRAM is None:
        _PROGRAM = _build_program()
    return _PROGRAM


def _tile_xT(x, nblk, width):
    # x [n, 1024] f32 -> x^T bf16 tiled [nblk, 128 p, 8 k, width]
    xt = np.ascontiguousarray(x.T).astype(NP_BF16)       # [1024, n]
    return np.ascontiguousarray(
        xt.reshape(KT, P, nblk, width).transpose(2, 1, 0, 3))


def _tile_w(w_rows):
    # w_rows [256, 1024] (= W[g-slice]) -> W^T bf16 tiled [128 p, 8 k, 256]
    return np.ascontiguousarray(
        w_rows.T.astype(NP_BF16).reshape(KT, P, DG).transpose(1, 0, 2))


def make_in_maps(queries, keys, values, Wq, bq, Wk, bk, Wv, bv, Wo, bo):
    """Build per-core input dicts. Core c handles batch c//4, head group c%4."""
    f32 = np.float32
    xT = {}
    for ib in range(B):
        xT[ib] = (
            _tile_xT(np.asarray(queries[ib], f32), NQB, QB),
            _tile_xT(np.asarray(keys[ib], f32), NQB, QB),
            _tile_xT(np.asarray(values[ib], f32), NKT, P),
        )
    in_maps = []
    for core in range(8):
        ib, g = core // G, core % G
        sl = slice(g * DG, (g + 1) * DG)
        in_maps.append({
            "xqT": xT[ib][0], "xkT": xT[ib][1], "xvT": xT[ib][2],
            "wqT": _tile_w(Wq[sl, :]),
            "wkT": _tile_w(Wk[sl, :]),
            "wvT": _tile_w(Wv[sl, :]),
            "woT": np.ascontiguousarray(
                Wo[:, sl].T.astype(NP_BF16).reshape(2, P, D_MODEL)
                .transpose(1, 0, 2)),
            "bq_s": np.ascontiguousarray(bq[sl]).astype(f32),
        })
    return in_maps


def gather_output(results, Wo, bv, bo):
    out = np.zeros((B, N, D_MODEL), np.float32)
    for core in range(8):
        out[core // G] += results[core]["yT"].T
    # K bias is softmax-invariant (per-query constant logit shift); V bias
    # commutes through normalized attention into a constant output offset.
    const = bo.astype(np.float64) + Wo.astype(np.float64) @ bv.astype(np.float64)
    out += const[None, None, :].astype(np.float32)
    return out


def _run(inputs, trace=False, **spmd_kwargs):
    nc = get_program()
    in_maps = make_in_maps(**inputs)
    res = run_bass_kernel_spmd(nc, in_maps, core_ids=list(range(8)),
                               trace=trace, **spmd_kwargs)
    return gather_output(res.results, inputs["Wo"], inputs["bv"],
                         inputs["bo"]), res


def kernel(**inputs) -> np.ndarray:
    inputs = {k: np.asarray(v, dtype=np.float32) for k, v in inputs.items()}
    out, _ = _run(inputs, trace=False)
    return out


# revision 69
# speedup vs baseline: 1.1803x; 1.0271x over previous
"""Multi-head attention (b=2, n=2048, d_model=1024, H=16, d_k=d_v=64) on 8
Trainium2 NeuronCores.

Sharding: 8 cores = 2 (batch) x 4 (head groups of 4 heads).  Each core
computes, for its batch ib and head group g (heads as 2 pairs x 2 hp):

    qT/kT projections   qT = Wq_g @ x^T            [256, 2048]
    v projection        V  = x @ Wv_g^T            [2048, 256]
    per head: S^T = K_h Q_h^T (kpos on partitions), A^T = exp(S^T/8),
              O^T|Z = [V_h|1]^T A^T  (PSUM row 64 gives softmax denom Z)
    normalize O^T by 1/Z (Z broadcast across partitions via a tiny
    ones-outer-product matmul on the PE -- no DRAM roundtrip),
    out-projection Y^T = Wo_g @ O_cat^T            [1024, 2048]

Host sums the 4 per-group partial Y^T per batch and adds bo + Wo@bv
(the V bias commutes through normalized attention; the K bias is a
per-query constant shift of the logits, to which softmax is exactly
invariant, so both are folded out of the device program).

All matmul operands are bf16 (PSUM accumulation stays fp32): this halves
input DMA, enables Fast Weight Load on the PE (2x faster LDWEIGHTS), and
doubles DVE throughput where it applies.  exp is evaluated on fp32 PSUM
logits.  Softmax skips max-subtraction: scores*scale are ~N(0,1).

The PE instruction stream is software-pipelined: within the attention
inner loop the AV matmuls lag the S matmuls by one k-tile so the ACT
(exp) stream is hidden, and projection / out-projection matmuls are
sprinkled between attention tiles as filler so the PE never idles long
enough for the HAM clock gate to re-throttle.
"""

import numpy as np
from contextlib import ExitStack

import ml_dtypes

import concourse.bass as bass
import concourse.mybir as mybir
import concourse.tile as tile
from concourse import bacc
from concourse.bass_utils import run_bass_kernel_spmd

F32 = mybir.dt.float32
BF16 = mybir.dt.bfloat16
NP_BF16 = ml_dtypes.bfloat16
EXP = mybir.ActivationFunctionType.Exp
ADD = mybir.AluOpType.add
MULT = mybir.AluOpType.mult

D_MODEL = 1024
H = 16
DK = 64
B = 2
N = 2048           # nq = nk
G = 4              # head groups (cores per batch)
HG = H // G        # heads per group = 4
DG = HG * DK       # 256 group dims
KT = 8             # D_MODEL / 128 contraction tiles
NKT = N // 128     # 16 k-tiles in attention
QB = 512           # attention q-block
NQB = N // QB      # 4 q-blocks
P = 128

_PROGRAM = None
DEBUG_DUMP = False


def _build_program():
    nc = bacc.Bacc("TRN2", target_bir_lowering=False, debug=False, num_devices=8)

    # pre-tiled on host (bf16): per-partition lines are contiguous reads
    xqT = nc.dram_tensor("xqT", [NQB, P, KT, QB], BF16, kind="ExternalInput").ap()
    xkT = nc.dram_tensor("xkT", [NQB, P, KT, QB], BF16, kind="ExternalInput").ap()
    xvT = nc.dram_tensor("xvT", [NKT, P, KT, P], BF16, kind="ExternalInput").ap()
    wqT = nc.dram_tensor("wqT", [P, KT, DG], BF16, kind="ExternalInput").ap()
    wkT = nc.dram_tensor("wkT", [P, KT, DG], BF16, kind="ExternalInput").ap()
    wvT = nc.dram_tensor("wvT", [P, KT, DG], BF16, kind="ExternalInput").ap()
    woT = nc.dram_tensor("woT", [P, 2, D_MODEL], BF16, kind="ExternalInput").ap()
    bq_d = nc.dram_tensor("bq_s", [DG], F32, kind="ExternalInput").ap()
    yT_d = nc.dram_tensor("yT", [D_MODEL, N], F32, kind="ExternalOutput").ap()
    if DEBUG_DUMP:
        dbg_kt = nc.dram_tensor("dbg_kt", [P, 2, N], F32, kind="ExternalOutput").ap()
        dbg_v = nc.dram_tensor("dbg_v", [P, NKT, HG, DK + 1], F32,
                               kind="ExternalOutput").ap()
        dbg_qt = nc.dram_tensor("dbg_qt", [P, 2, QB], F32, kind="ExternalOutput").ap()
        dbg_o = nc.dram_tensor("dbg_o", [P, 2, QB], F32, kind="ExternalOutput").ap()
        dbg_at = nc.dram_tensor("dbg_at", [P, 2, QB], F32, kind="ExternalOutput").ap()
        dbg_av = nc.dram_tensor("dbg_av", [P, QB], F32, kind="ExternalOutput").ap()
        dbg_bc = nc.dram_tensor("dbg_bc", [P, QB], F32, kind="ExternalOutput").ap()

    bq_v = bq_d.rearrange("(j p) -> p j", p=P)        # [128, 2]

    with tile.TileContext(nc) as tc:
        with ExitStack() as ctx:
            const = ctx.enter_context(tc.tile_pool(name="const", bufs=1))
            xin = ctx.enter_context(tc.tile_pool(name="xin", bufs=2))
            xvp = ctx.enter_context(tc.tile_pool(name="xvp", bufs=3))
            atp = ctx.enter_context(tc.tile_pool(name="atp", bufs=4))
            smal = ctx.enter_context(tc.tile_pool(name="smal", bufs=2))
            stp = ctx.enter_context(tc.tile_pool(name="stp", bufs=2, space="PSUM"))
            avp = ctx.enter_context(tc.tile_pool(name="avp", bufs=2, space="PSUM"))
            aux = ctx.enter_context(tc.tile_pool(name="aux", bufs=2, space="PSUM"))

            # ---- constants: weights on the ACT queue (idle at start),
            # x chunks on SP, xv on DVE ----
            wk_sb = const.tile([P, KT, DG], BF16, tag="wk")
            wv_sb = const.tile([P, KT, DG], BF16, tag="wv")
            wq_sb = const.tile([P, KT, DG], BF16, tag="wq")
            wo_sb = const.tile([P, 2, D_MODEL], BF16, tag="wo")
            nc.sync.dma_start(wk_sb[:], wkT)  # first: gates the K projections
            nc.scalar.dma_start(wv_sb[:], wvT)
            nc.scalar.dma_start(wq_sb[:], wqT)
            nc.scalar.dma_start(wo_sb[:], woT)
            bq_sb = const.tile([P, 2], F32, tag="bq")
            nc.scalar.dma_start(bq_sb[:], bq_v)

            # K^T stored zero-padded per head-half: kt2_sb[:, hp, pair, n]
            # has K_h dims on partitions hp*64..hp*64+64 and ZEROS on the
            # other 64 partitions, so S matmuls contract over the full 128
            # partitions in (128,128) array mode — no tiling-mode switches
            # (each 64<->128 mode change drains the tensor engine).
            kt2_sb = const.tile([P, 2, 2, N], BF16, tag="kt2")
            nc.vector.memset(kt2_sb.rearrange("p a b c -> p (a b c)"), 0.0)
            v_sb = const.tile([P, NKT, HG, DK + 1], BF16, tag="v")  # [V_h | 1]
            # whole-tile memset to 1.0; V-proj copies overwrite cols 0:64,
            # leaving the denominator ones-column.  (A strided ones DMA here
            # generates thousands of 2-byte descriptors and stalls the DMA
            # engines for ~50us.)
            nc.vector.memset(
                v_sb.rearrange("p a b c -> p (a b c)"), 1.0)

            qts = {}
            o_sb = {}
            for c in range(NQB):
                qts[c] = const.tile([P, 2, QB], BF16, tag=f"qt{c}", name=f"qt_{c}")
                o_sb[c] = const.tile([P, 2, QB], BF16, tag=f"o{c}", name=f"o_{c}")

            # x DMAs up front; xk0 + xq0 first so the head (K0 + Q0
            # projections) is never queued behind the xv stream
            xks, xqs, xvs = {}, {}, {}

            def load_xk(c):
                xks[c] = xin.tile([P, KT, QB], BF16, tag="xk", name=f"xk_{c}",
                                  bufs=4)
                nc.sync.dma_start(xks[c][:], xkT[c])

            def load_xq(c):
                xqs[c] = xin.tile([P, KT, QB], BF16, tag="xq", name=f"xq_{c}",
                                  bufs=4)
                nc.sync.dma_start(xqs[c][:], xqT[c])

            for c in range(NQB):
                load_xk(c)
            load_xq(0)
            for nt in range(NKT):
                xvs[nt] = xvp.tile([P, KT, P], BF16, tag="xv", name=f"xv_{nt}",
                                   bufs=8)
                nc.sync.dma_start(xvs[nt][:], xvT[nt])
            for c in range(1, NQB):
                load_xq(c)

            # ---------------- emission helpers ----------------
            def k_proj(c, j):
                ps = aux.tile([P, QB], F32, tag="aux", name=f"kps_{c}_{j}")
                for k in range(KT):
                    nc.tensor.matmul(
                        ps[:], wk_sb[:, k, j * P:(j + 1) * P], xks[c][:, k, :],
                        start=(k == 0), stop=(k == KT - 1))
                sl = slice(c * QB, (c + 1) * QB)
                nc.vector.tensor_copy(kt2_sb[0:DK, 0, j, sl], ps[0:DK, :])
                nc.vector.tensor_copy(kt2_sb[DK:P, 1, j, sl], ps[DK:P, :])

            qps_live = {}

            def q_proj_quarter(c, j, quarter):
                # 2-matmul pieces: small enough to hide in the per-tile PE
                # lead so popping one never slips the exp stream
                if quarter == 0:
                    qps_live[c, j] = aux.tile([P, QB], F32, tag="aux",
                                              name=f"qps_{c}_{j}")
                ps = qps_live[c, j]
                for k in range(quarter * 2, quarter * 2 + 2):
                    nc.tensor.matmul(
                        ps[:], wq_sb[:, k, j * P:(j + 1) * P], xqs[c][:, k, :],
                        start=(k == 0), stop=(k == KT - 1))
                if quarter == 3:
                    nc.vector.tensor_tensor(
                        qts[c][:, j, :], ps[:],
                        bq_sb[:, j, None].to_broadcast((P, QB)), ADD)
                    del qps_live[c, j]

            def q_proj(c, j):
                for quarter in range(4):
                    q_proj_quarter(c, j, quarter)

            vps_live = {}

            def v_proj_half(nt, half):
                if half == 0:
                    vps_live[nt] = aux.tile([P, QB], F32, tag="aux",
                                            name=f"vps_{nt}")
                ps = vps_live[nt]
                for k in range(half * 4, half * 4 + 4):
                    nc.tensor.matmul(ps[:, 0:DG], xvs[nt][:, k, :], wv_sb[:, k, :],
                                     start=(k == 0), stop=(k == KT - 1))
                if half == 1:
                    nc.vector.tensor_copy(
                        v_sb[:, nt, :, 0:DK],
                        ps[:, 0:DG].rearrange("p (h d) -> p h d", h=HG))
                    del vps_live[nt]

            def o_proj(c, m):
                yps = aux.tile([P, QB], F32, tag="aux", name=f"yps_{c}_{m}")
                for j in range(2):
                    nc.tensor.matmul(
                        yps[:], wo_sb[:, j, m * P:(m + 1) * P], o_sb[c][:, j, :],
                        start=(j == 0), stop=(j == 1))
                y_sb = smal.tile([P, QB], F32, tag="y", name=f"y_{c}_{m}",
                                 bufs=8)
                if c == NQB - 1:
                    # tail only: ACT is idle there, DVE runs the normalize
                    nc.scalar.copy(y_sb[:], yps[:])
                else:
                    nc.vector.tensor_copy(y_sb[:], yps[:])
                # all output DMAs on the (idle) SP queue: SWDGE generation on
                # the gpsimd queue jams the Pool engine ahead of the softmax
                # broadcasts and stalls the tail ~9us
                nc.sync.dma_start(
                    yT_d[m * P:(m + 1) * P, c * QB:(c + 1) * QB], y_sb[:])

            # filler queue: PE work units (projections, out-projections)
            # interleaved into the exp-paced attention loop.  Forced drains
            # keep dependencies ahead: K chunk kt//4 before S(kt), V tile kt
            # before AV(kt), Q chunk c before attention(c).
            state = {"k": NQB, "v": 0, "q": 1}
            filler = []
            for nt in range(NKT):
                filler.append(("v", nt, lambda nt=nt: v_proj_half(nt, 0)))
                filler.append(("v", nt + 1, lambda nt=nt: v_proj_half(nt, 1)))
            for c in range(1, NQB):
                for j in range(2):
                    for quarter in range(4):
                        idx = c if (j, quarter) != (1, 3) else c + 1
                        filler.append(("q", idx,
                                       lambda c=c, j=j, q4=quarter:
                                       q_proj_quarter(c, j, q4)))

            def pop_filler(k=1):
                for _ in range(k):
                    if filler:
                        kind, idx, fn = filler.pop(0)
                        fn()
                        if kind in state:
                            state[kind] = max(state[kind], idx)

            def drain_until(kind, idx):
                while state.get(kind, idx) < idx and filler:
                    pop_filler()

            def emit_av(c, pair, avs, ats, kt):
                drain_until("v", kt + 1)
                for hp in range(2):
                    h = 2 * pair + hp
                    nc.tensor.matmul(
                        avs[hp][:], v_sb[:, kt, h, :], ats[kt][:, hp, :],
                        start=(kt == 0), stop=(kt == NKT - 1))

            # ---- head: all K chunks + Q chunk 0; V and later Q chunks
            # drain as filler inside the exp-paced attention loop ----
            for c4 in range(NQB):
                k_proj(c4, 0)
                k_proj(c4, 1)
            q_proj(0, 0)
            q_proj(0, 1)

            # ---- attention ----
            dbg_at_sb = {}

            def close_pair(c, pair, avs, ats):
                """Last AV + accumulator evacuation + softmax normalization
                for (c, pair).  Deferred into the NEXT pair's loop so the
                boundary never stalls the exp stream."""
                emit_av(c, pair, avs, ats, NKT - 1)
                if DEBUG_DUMP and c == 0 and pair == 0:
                    snap_av = const.tile([P, QB], F32, tag="dbgav")
                    nc.vector.tensor_copy(snap_av[0:DK + 1, :],
                                          avs[0][0:DK + 1, :])
                    dbg_at_sb["av"] = snap_av
                # evacuate the accumulators to SBUF immediately: one DVE
                # copy frees the PSUM slot so the next pair's AV matmuls
                # don't wait for the (slow) normalization chain.  The very
                # last pair skips evacuation (no successor) and multiplies
                # straight from PSUM off the tail's critical path.
                last = (c == NQB - 1 and pair == 1)
                avsb = {}
                zrs = {}
                for hp in range(2):
                    zrs[hp] = smal.tile([1, QB], F32, tag=f"zr{hp}",
                                        name=f"zr_{c}_{pair}_{hp}")
                    nc.vector.tensor_copy(zrs[hp][:], avs[hp][DK:DK + 1, :])
                    if last:
                        avsb[hp] = avs[hp][0:DK, :]
                    else:
                        avsb[hp] = smal.tile([DK, QB], F32, tag=f"avsb{hp}",
                                             name=f"avsb_{c}_{pair}_{hp}")
                        nc.vector.tensor_copy(avsb[hp][:], avs[hp][0:DK, :])
                zbcs = {}
                for hp in range(2):
                    rzf = smal.tile([1, QB], F32, tag="rzf",
                                    name=f"rzf_{c}_{pair}_{hp}")
                    nc.vector.reciprocal_approx_fast(rzf[:], zrs[hp][:])
                    zbcs[hp] = smal.tile([DK, QB], F32, tag=f"zbc{hp}",
                                         name=f"zbc_{c}_{pair}_{hp}")
                    nc.gpsimd.partition_broadcast(zbcs[hp][:], rzf[:],
                                                  channels=DK)
                for hp in range(2):
                    if DEBUG_DUMP and c == 0 and pair == 0 and hp == 0:
                        snap_bc = const.tile([P, QB], F32, tag="dbgbc")
                        nc.vector.tensor_copy(snap_bc[0:DK, :], zbcs[0][:])
                        dbg_at_sb["bc"] = snap_bc
                    nc.vector.tensor_tensor(
                        o_sb[c][DK * hp:DK * (hp + 1), pair, :],
                        avsb[hp][:], zbcs[hp][:], MULT)
                if pair == 1:
                    # chunk done: its out-projection becomes filler
                    for m in range(8):
                        filler.append(("o", 0, lambda c=c, m=m: o_proj(c, m)))

            # flat step schedule; each step emits the NEXT step's S matmuls
            # first so the S stream leads the PE queue and the exp gate for
            # step i+1 is already satisfied when exp(i) finishes
            steps = [(c, pair, kt)
                     for c in range(NQB) for pair in range(2)
                     for kt in range(NKT)]
            stbs = {}

            def emit_s(i):
                c, pair, kt = steps[i]
                if pair == 0 and kt == 0:
                    drain_until("q", c + 1)
                stb = stp.tile([P, 2, QB], F32, tag="st",
                               name=f"st_{c}_{pair}_{kt}")
                for hp in range(2):
                    nc.tensor.matmul(
                        stb[:, hp, :],
                        kt2_sb[:, hp, pair, kt * P:(kt + 1) * P],
                        qts[c][:, pair, :],
                        start=True, stop=True)
                stbs[i] = stb

            pending_close = None
            avs = None
            ats = {}
            emit_s(0)
            for i, (c, pair, kt) in enumerate(steps):
                if kt == 0:
                    avs = [avp.tile([DK + 1, QB], F32, tag="av",
                                    name=f"av_{c}_{pair}_{hp}")
                           for hp in range(2)]
                    ats = {}
                if i + 1 < len(steps):
                    emit_s(i + 1)
                at = atp.tile([P, 2, QB], BF16, tag="at",
                              name=f"at_{c}_{pair}_{kt}")
                nc.scalar.activation(
                    at.rearrange("p a b -> p (a b)"),
                    stbs.pop(i).rearrange("p a b -> p (a b)"), EXP, scale=0.125)
                ats[kt] = at
                if DEBUG_DUMP and c == 0 and pair == 0 and kt == 0:
                    snap = const.tile([P, 2, QB], F32, tag="dbgsnap")
                    nc.vector.tensor_copy(
                        snap.rearrange("p a b -> p (a b)"),
                        at.rearrange("p a b -> p (a b)"))
                    dbg_at_sb[0] = snap
                if kt == 0 and pending_close is not None:
                    pending_close()
                    pending_close = None
                # AV lags one k-tile so exp latency is hidden
                if kt > 0:
                    emit_av(c, pair, avs, ats, kt - 1)
                # rate-limit cadence pops: exp-stream slips are permanent
                # (the exp period cannot compress), so fillers must be
                # spaced against the ~200ns/kt PE lead rather than popped
                # greedily in chunk-start bursts
                if kt % 3 == 2:
                    pop_filler()
                if kt == NKT - 1:
                    pending_close = (lambda c=c, pair=pair, avs=avs, ats=ats:
                                     close_pair(c, pair, avs, ats))
            pending_close()

            # drain remaining filler (last chunk's out-projection)
            pop_filler(len(filler))

            if DEBUG_DUMP:
                for nm, sb, dr in (("kt", kt_sb, dbg_kt), ("v", v_sb, dbg_v),
                                   ("qt", qts[0], dbg_qt), ("o", o_sb[0], dbg_o),
                                   ("at", dbg_at_sb[0], dbg_at)):
                    f = smal.tile(list(sb.shape), F32, tag=f"dbg{nm}",
                                  name=f"dbgt_{nm}")
                    nc.vector.tensor_copy(
                        f.rearrange("p a b c -> p (a b c)") if len(sb.shape) == 4
                        else f.rearrange("p a b -> p (a b)"),
                        sb.rearrange("p a b c -> p (a b c)") if len(sb.shape) == 4
                        else sb.rearrange("p a b -> p (a b)"))
                    nc.sync.dma_start(dr, f[:])
                nc.sync.dma_start(dbg_av, dbg_at_sb["av"][:])
                nc.sync.dma_start(dbg_bc, dbg_at_sb["bc"][:])

    nc.compile()
    return nc


def get_program():
    global _PROGRAM
    if _PROGRAM is None:
        _PROGRAM = _build_program()
    return _PROGRAM


def _tile_xT(x, nblk, width):
    # x [n, 1024] f32 -> x^T bf16 tiled [nblk, 128 p, 8 k, width]
    xt = np.ascontiguousarray(x.T).astype(NP_BF16)       # [1024, n]
    return np.ascontiguousarray(
        xt.reshape(KT, P, nblk, width).transpose(2, 1, 0, 3))


def _tile_w(w_rows):
    # w_rows [256, 1024] (= W[g-slice]) -> W^T bf16 tiled [128 p, 8 k, 256]
    return np.ascontiguousarray(
        w_rows.T.astype(NP_BF16).reshape(KT, P, DG).transpose(1, 0, 2))


def make_in_maps(queries, keys, values, Wq, bq, Wk, bk, Wv, bv, Wo, bo):
    """Build per-core input dicts. Core c handles batch c//4, head group c%4."""
    f32 = np.float32
    xT = {}
    for ib in range(B):
        xT[ib] = (
            _tile_xT(np.asarray(queries[ib], f32), NQB, QB),
            _tile_xT(np.asarray(keys[ib], f32), NQB, QB),
            _tile_xT(np.asarray(values[ib], f32), NKT, P),
        )
    in_maps = []
    for core in range(8):
        ib, g = core // G, core % G
        sl = slice(g * DG, (g + 1) * DG)
        in_maps.append({
            "xqT": xT[ib][0], "xkT": xT[ib][1], "xvT": xT[ib][2],
            "wqT": _tile_w(Wq[sl, :]),
            "wkT": _tile_w(Wk[sl, :]),
            "wvT": _tile_w(Wv[sl, :]),
            "woT": np.ascontiguousarray(
                Wo[:, sl].T.astype(NP_BF16).reshape(2, P, D_MODEL)
                .transpose(1, 0, 2)),
            "bq_s": np.ascontiguousarray(bq[sl]).astype(f32),
        })
    return in_maps


def gather_output(results, Wo, bv, bo):
    out = np.zeros((B, N, D_MODEL), np.float32)
    for core in range(8):
        out[core // G] += results[core]["yT"].T
    # K bias is softmax-invariant (per-query constant logit shift); V bias
    # commutes through normalized attention into a constant output offset.
    const = bo.astype(np.float64) + Wo.astype(np.float64) @ bv.astype(np.float64)
    out += const[None, None, :].astype(np.float32)
    return out


def _run(inputs, trace=False, **spmd_kwargs):
    nc = get_program()
    in_maps = make_in_maps(**inputs)
    res = run_bass_kernel_spmd(nc, in_maps, core_ids=list(range(8)),
                               trace=trace, **spmd_kwargs)
    return gather_output(res.results, inputs["Wo"], inputs["bv"],
                         inputs["bo"]), res


def kernel(**inputs) -> np.ndarray:
    inputs = {k: np.asarray(v, dtype=np.float32) for k, v in inputs.items()}
    out, _ = _run(inputs, trace=False)
    return out
